# revision 3
# baseline (speedup 1.0000x reference)
import os
import sys
import numpy as np

sys.path.insert(0, "/opt/trn_rl_repo")

TRACE = bool(os.environ.get("KTRACE"))
EXEC_NS = []
TRACE_PATHS = []

N = 20000
NP = 20480          # padded node count (160 tiles of 128)
E = 320000
H = 4
D = 64
HID = 256
EMB = 64
FIN = 128
OUT = 8
SRC_T = (0, 2, 2, 0, 1, 1, 0, 1, 2)
DST_T = (1, 1, 0, 0, 2, 0, 0, 1, 2)
NCORES = 8
PT = NP // NCORES   # dst rows per core (2560)
SENT = NP           # sentinel gather row


# ---------------------------------------------------------------- numpy ref

def _gat_layer_np(h, W, al, ar, b, edges):
    out = np.zeros((3, N, H, D), np.float32)
    for e in range(9):
        st, dt = SRC_T[e], DST_T[e]
        src, dst = edges[e, 0], edges[e, 1]
        z_src = (h[st] @ W[e]).reshape(N, H, D)
        z_dst = (h[dt] @ W[e]).reshape(N, H, D)
        el = np.sum(z_src * al[e], axis=-1)
        er = np.sum(z_dst * ar[e], axis=-1)
        s = el[src] + er[dst]
        s = np.where(s > 0, s, 0.2 * s)
        m = np.full((N, H), -np.inf, np.float32)
        np.maximum.at(m, dst, s)
        ex = np.exp(s - np.where(np.isfinite(m[dst]), m[dst], 0.0))
        den = np.zeros((N, H), np.float32)
        np.add.at(den, dst, ex)
        alpha = ex / den[dst]
        agg = np.zeros((N, H, D), np.float32)
        np.add.at(agg, dst, alpha[:, :, None] * z_src[src])
        out[dt] += agg + b[e].reshape(H, D)
    return out


def _ln_relu(v, gamma, beta):
    mu = v.mean(-1, keepdims=True)
    var = v.var(-1, keepdims=True)
    v = (v - mu) / np.sqrt(var + 1e-5) * gamma[:, None, :] + beta[:, None, :]
    return np.maximum(v, 0.0)


def _kernel_np(x, edges, Wemb, bemb, W0, al0, ar0, b0, W1, al1, ar1, b1,
               gamma, beta, Wc, bc):
    h = np.einsum('tnf,tfe->tne', x, Wemb) + bemb[:, None, :]
    h = _ln_relu(_gat_layer_np(h, W0, al0, ar0, b0, edges).reshape(3, N, HID),
                 gamma, beta)
    for l in range(2):
        h = _ln_relu(_gat_layer_np(h, W1[l], al1[l], ar1[l], b1[l],
                                   edges).reshape(3, N, HID), gamma, beta)
    return np.einsum('tnh,tho->tno', h, Wc) + bc[:, None, :]


# ---------------------------------------------------------------- edge prep

def _prep_edges(edges):
    """Per edge type: sort by dst, bucket into 128-dst-node tiles, pad each
    tile's edge list to CH chunks of 128. Returns (CH, srcidx, dstl) with
    srcidx [9, NP, CH] int32 (SENT pad) and dstl [9, NP, CH] f32 (999 pad),
    laid out so edge j of a tile sits at [tile*128 + j%128, j//128]."""
    counts = np.zeros((9, NP // 128), np.int64)
    order_all, dst_all, src_all = [], [], []
    for e in range(9):
        src, dst = edges[e, 0].astype(np.int64), edges[e, 1].astype(np.int64)
        o = np.argsort(dst, kind="stable")
        src, dst = src[o], dst[o]
        order_all.append(o); dst_all.append(dst); src_all.append(src)
        counts[e] = np.bincount(dst // 128, minlength=NP // 128)
    CH = int(np.ceil(counts.max() / 128))
    srcidx = np.full((9, NP, CH), SENT, np.int32)
    dstl = np.full((9, NP, CH), 999.0, np.float32)
    for e in range(9):
        dst, src = dst_all[e], src_all[e]
        tile = dst // 128
        starts = np.zeros(NP // 128, np.int64)
        starts[1:] = np.cumsum(counts[e])[:-1]
        rank = np.arange(E) - starts[tile]
        row = tile * 128 + rank % 128
        col = rank // 128
        srcidx[e, row, col] = src
        dstl[e, row, col] = (dst - tile * 128).astype(np.float32)
    return CH, srcidx, dstl


# ---------------------------------------------------------------- bass prog

def _build_program(Din, CH, skip_a=False, nb=9):
    from concourse import bass, mybir, tile
    from concourse.bass import ds

    f32 = mybir.dt.float32
    i32 = mybir.dt.int32
    KC = max(1, Din // 128)
    kcw = Din // KC
    ZROWS = 9 * (NP + 1)
    GT = 9 * PT

    nc = bass.Bass()
    hT = nc.dram_tensor("ht", [3, KC, kcw, NP], f32, kind="ExternalInput")
    W9 = nc.dram_tensor("w9", [9, KC, kcw, 260], f32, kind="ExternalInput")
    ERI = nc.dram_tensor("eri", [GT, 4], f32, kind="ExternalInput")
    SRC = nc.dram_tensor("srci", [GT, CH], i32, kind="ExternalInput")
    DSTL = nc.dram_tensor("dstl", [GT, CH], f32, kind="ExternalInput")
    IOTAF = nc.dram_tensor("iotaf", [128, 128], f32, kind="ExternalInput")
    IDENT = nc.dram_tensor("ident", [128, 128], f32, kind="ExternalInput")
    ROWIX = nc.dram_tensor("rowix", [GT, 1], i32, kind="ExternalInput")
    ROWA = nc.dram_tensor("rowa", [NP, 1], i32, kind="ExternalInput")
    SENTIX = nc.dram_tensor("sentix", [9, 1], i32, kind="ExternalInput")
    OUTE = nc.dram_tensor("oute", [GT, 256], f32, kind="ExternalOutput")
    ZELB = nc.dram_tensor("zelb", [ZROWS, 260], f32)

    with tile.TileContext(nc) as tc:
        import contextlib
        with contextlib.ExitStack() as ctx:
            singles = ctx.enter_context(tc.tile_pool(name="singles", bufs=24))
            iotaf_sb = singles.tile([128, 128], f32)
            ident_sb = singles.tile([128, 128], f32)
            nc.sync.dma_start(out=iotaf_sb[:], in_=IOTAF[:])
            nc.sync.dma_start(out=ident_sb[:], in_=IDENT[:])
            w_sb = []
            for e in range(9):
                per_kc = []
                for kc in range(KC):
                    wt = singles.tile([kcw, 260], f32)
                    nc.sync.dma_start(out=wt[:], in_=W9[e][kc])
                    per_kc.append(wt)
                w_sb.append(per_kc)

            # sentinel rows: z = 0, el = -1e4, scattered to e*(NP+1)+NP
            sent_sb = singles.tile([9, 260], f32)
            nc.vector.memset(sent_sb[:], 0.0)
            nc.vector.memset(sent_sb[:, 256:260], -1.0e4)
            sentix_sb = singles.tile([9, 1], i32)
            nc.sync.dma_start(out=sentix_sb[:], in_=SENTIX[:])
            nc.gpsimd.indirect_dma_start(
                out=ZELB[:, :],
                out_offset=bass.IndirectOffsetOnAxis(ap=sentix_sb[:, :1],
                                                     axis=0),
                in_=sent_sb[:], in_offset=None)

            pa = ctx.enter_context(tc.tile_pool(name="pa", bufs=3))
            pah = ctx.enter_context(tc.tile_pool(name="pah", bufs=2))
            paz = ctx.enter_context(tc.tile_pool(name="paz", bufs=3))
            par = ctx.enter_context(tc.tile_pool(name="par", bufs=3))
            pap = ctx.enter_context(tc.tile_pool(name="pap", bufs=1,
                                                 space="PSUM"))
            pb = ctx.enter_context(tc.tile_pool(name="pb", bufs=2))
            ptp = ctx.enter_context(tc.tile_pool(name="ptp", bufs=2,
                                                 space="PSUM"))
            pep = ctx.enter_context(tc.tile_pool(name="pep", bufs=1,
                                                 space="PSUM"))
            pdp = ctx.enter_context(tc.tile_pool(name="pdp", bufs=1,
                                                 space="PSUM"))

            # ---- phase A: dense projections z|el -> ZELB (scatter rows)
            if not skip_a:
                with tc.For_i(0, NP, 128) as r0:
                    rsb = pa.tile([128, 1], i32)
                    nc.sync.dma_start(out=rsb[:], in_=ROWA[ds(r0, 128), :])
                    h_sb = []
                    for t in range(3):
                        per_kc = []
                        for kc in range(KC):
                            htile = pah.tile([kcw, 128], f32)
                            nc.sync.dma_start(
                                out=htile[:], in_=hT[t][kc][:, ds(r0, 128)])
                            per_kc.append(htile)
                        h_sb.append(per_kc)
                    for e in range(9):
                        ps = pap.tile([128, 260], f32, space="PSUM")
                        for kc in range(KC):
                            nc.tensor.matmul(
                                out=ps[:], lhsT=h_sb[SRC_T[e]][kc][:],
                                rhs=w_sb[e][kc][:],
                                start=(kc == 0), stop=(kc == KC - 1))
                        zsb = paz.tile([128, 260], f32)
                        nc.vector.tensor_copy(out=zsb[:], in_=ps[:])
                        ridx = par.tile([128, 1], i32)
                        nc.vector.tensor_scalar_add(ridx[:], rsb[:],
                                                    e * (NP + 1))
                        nc.gpsimd.indirect_dma_start(
                            out=ZELB[:, :],
                            out_offset=bass.IndirectOffsetOnAxis(
                                ap=ridx[:, :1], axis=0),
                            in_=zsb[:], in_offset=None)

            # ---- phase B: per-edge softmax + aggregation, all 9 types in
            # one hardware loop over GT = 9*PT rows
            if nb:
                with tc.For_i(0, GT, 128) as t0:
                    dstl_sb = pb.tile([128, CH], f32)
                    srci_sb = pb.tile([128, CH], i32)
                    er_sb = pb.tile([128, 4], f32)
                    nc.sync.dma_start(out=dstl_sb[:],
                                      in_=DSTL[ds(t0, 128), :])
                    nc.sync.dma_start(out=srci_sb[:],
                                      in_=SRC[ds(t0, 128), :])
                    nc.sync.dma_start(out=er_sb[:], in_=ERI[ds(t0, 128), :])

                    zel_sb = pb.tile([128, CH, 260], f32)
                    for c in range(CH):
                        nc.gpsimd.indirect_dma_start(
                            out=zel_sb[:, c, :], out_offset=None,
                            in_=ZELB[:, :],
                            in_offset=bass.IndirectOffsetOnAxis(
                                ap=srci_sb[:, c:c + 1], axis=0))

                    Mt = pb.tile([128, CH, 128], f32)
                    Ma = pb.tile([128, CH, 128], f32)
                    for c in range(CH):
                        nc.vector.tensor_tensor(
                            out=Mt[:, c, :],
                            in0=dstl_sb[:, c:c + 1].to_broadcast([128, 128]),
                            in1=iotaf_sb[:],
                            op=mybir.AluOpType.is_equal)
                    for c in range(CH):
                        tp = ptp.tile([128, 128], f32, space="PSUM")
                        nc.tensor.transpose(out=tp[:], in_=Mt[:, c, :],
                                            identity=ident_sb[:])
                        nc.vector.tensor_copy(out=Ma[:, c, :], in_=tp[:])

                    erx = pb.tile([128, CH, 4], f32)
                    for c in range(CH):
                        ep = pep.tile([128, 4], f32, space="PSUM")
                        nc.tensor.matmul(out=ep[:], lhsT=Ma[:, c, :],
                                         rhs=er_sb[:], start=True, stop=True)
                        nc.vector.tensor_copy(out=erx[:, c, :], in_=ep[:])

                    ex = pb.tile([128, CH, 4], f32)
                    nc.vector.tensor_tensor(
                        out=ex[:], in0=zel_sb[:, :, 256:260], in1=erx[:],
                        op=mybir.AluOpType.add)
                    nc.scalar.activation(
                        out=ex[:], in_=ex[:],
                        func=mybir.ActivationFunctionType.Lrelu, alpha=0.2)
                    nc.scalar.activation(
                        out=ex[:], in_=ex[:],
                        func=mybir.ActivationFunctionType.Exp)

                    den_ps = pap.tile([128, 4], f32, space="PSUM")
                    for c in range(CH):
                        nc.tensor.matmul(out=den_ps[:], lhsT=Mt[:, c, :],
                                         rhs=ex[:, c, :], start=(c == 0),
                                         stop=(c == CH - 1))
                    den_sb = pb.tile([128, 4], f32)
                    nc.vector.tensor_copy(out=den_sb[:], in_=den_ps[:])

                    dex = pb.tile([128, CH, 4], f32)
                    for c in range(CH):
                        dp = pdp.tile([128, 4], f32, space="PSUM")
                        nc.tensor.matmul(out=dp[:], lhsT=Ma[:, c, :],
                                         rhs=den_sb[:], start=True, stop=True)
                        nc.vector.tensor_copy(out=dex[:, c, :], in_=dp[:])
                    nc.vector.tensor_scalar_add(dex[:], dex[:], 1e-30)
                    alp = pb.tile([128, CH, 4], f32)
                    nc.vector.tensor_tensor(
                        out=alp[:], in0=ex[:], in1=dex[:],
                        op=mybir.AluOpType.divide)

                    vsc = pb.tile([128, CH, 256], f32)
                    for c in range(CH):
                        for hh in range(H):
                            nc.vector.tensor_tensor(
                                out=vsc[:, c, hh * D:(hh + 1) * D],
                                in0=zel_sb[:, c, hh * D:(hh + 1) * D],
                                in1=alp[:, c, hh:hh + 1].to_broadcast(
                                    [128, D]),
                                op=mybir.AluOpType.mult)

                    agg_ps = pap.tile([128, 256], f32, space="PSUM")
                    for c in range(CH):
                        nc.tensor.matmul(out=agg_ps[:], lhsT=Mt[:, c, :],
                                         rhs=vsc[:, c, :], start=(c == 0),
                                         stop=(c == CH - 1))
                    agg_sb = pb.tile([128, 256], f32)
                    nc.vector.tensor_copy(out=agg_sb[:], in_=agg_ps[:])
                    ridx = pb.tile([128, 1], i32)
                    nc.sync.dma_start(out=ridx[:], in_=ROWIX[ds(t0, 128), :])
                    nc.gpsimd.indirect_dma_start(
                        out=OUTE[:, :],
                        out_offset=bass.IndirectOffsetOnAxis(
                            ap=ridx[:, :1], axis=0),
                        in_=agg_sb[:], in_offset=None)
    return nc


_PROG_CACHE = {}


def _run_layer(h, W, al, ar, CH, srcidx, dstl):
    """One GAT layer on 8 cores. h [3,N,Din] f32. Returns pre-bias
    aggregated output [3, N, 256] (sum over edge types into dst type)."""
    from concourse.bass_utils import run_bass_kernel_spmd

    Din = h.shape[2]
    KC = max(1, Din // 128)
    kcw = Din // KC

    ALm = np.zeros((9, HID, H), np.float32)
    ARm = np.zeros((9, HID, H), np.float32)
    for e in range(9):
        for hh in range(H):
            ALm[e, hh * D:(hh + 1) * D, hh] = al[e, hh]
            ARm[e, hh * D:(hh + 1) * D, hh] = ar[e, hh]

    w9 = np.zeros((9, KC, kcw, 260), np.float32)
    for e in range(9):
        wext = np.concatenate([W[e], W[e] @ ALm[e]], axis=1)  # [Din, 260]
        w9[e] = wext.reshape(KC, kcw, 260)

    hTp = np.zeros((3, KC, kcw, NP), np.float32)
    for t in range(3):
        hTp[t, :, :, :N] = h[t].T.reshape(KC, kcw, N)

    er = np.zeros((9, NP, 4), np.float32)
    for e in range(9):
        er[e, :N] = h[DST_T[e]] @ (W[e] @ ARm[e])

    iotaf = np.broadcast_to(np.arange(128, dtype=np.float32),
                            (128, 128)).copy()
    ident = np.eye(128, dtype=np.float32)
    rowa = np.arange(NP, dtype=np.int32).reshape(NP, 1)
    sentix = (np.arange(9, dtype=np.int32) * (NP + 1) + NP).reshape(9, 1)
    GT = 9 * PT
    rowix = np.arange(GT, dtype=np.int32).reshape(GT, 1)

    key = (Din, CH)
    if key not in _PROG_CACHE:
        _PROG_CACHE[key] = _build_program(Din, CH)
    nc = _PROG_CACHE[key]

    # srcidx values get the per-edge-type ZELB base offset baked in; the
    # sentinel (SENT) maps to each type's sentinel row e*(NP+1)+NP
    in_maps = []
    for c in range(NCORES):
        sl = slice(c * PT, (c + 1) * PT)
        src2 = srcidx[:, sl].astype(np.int64)  # [9, PT, CH]
        base = (np.arange(9, dtype=np.int64) * (NP + 1))[:, None, None]
        src2 = np.where(src2 == SENT, NP, src2) + base
        in_maps.append({
            "ht": hTp, "w9": w9,
            "eri": np.ascontiguousarray(er[:, sl]).reshape(GT, 4),
            "srci": src2.reshape(GT, CH).astype(np.int32),
            "dstl": np.ascontiguousarray(dstl[:, sl]).reshape(GT, CH),
            "iotaf": iotaf, "ident": ident,
            "rowix": rowix, "rowa": rowa, "sentix": sentix,
        })
    res = run_bass_kernel_spmd(nc, in_maps, list(range(NCORES)), trace=TRACE)
    if res.exec_time_ns is not None:
        EXEC_NS.append(res.exec_time_ns)
    if res.instructions_and_trace is not None:
        TRACE_PATHS.append(res.instructions_and_trace[1])
    oute = np.stack([r["oute"].reshape(9, PT, 256) for r in res.results])
    out = np.zeros((3, N, HID), np.float32)
    for e in range(9):
        full = np.concatenate([oute[c, e] for c in range(NCORES)], axis=0)
        out[DST_T[e]] += full[:N]
    return out


def kernel(x, edges, Wemb, bemb, W0, al0, ar0, b0, W1, al1, ar1, b1,
           gamma, beta, Wc, bc):
    x = np.asarray(x, np.float32)
    edges = np.asarray(edges)
    args = [np.asarray(a, np.float32) for a in
            (Wemb, bemb, W0, al0, ar0, b0, W1, al1, ar1, b1, gamma, beta,
             Wc, bc)]
    Wemb, bemb, W0, al0, ar0, b0, W1, al1, ar1, b1, gamma, beta, Wc, bc = args
    try:
        CH, srcidx, dstl = _prep_edges(edges)
        h = np.einsum('tnf,tfe->tne', x, Wemb) + bemb[:, None, :]
        layers = [(W0, al0, ar0, b0), (W1[0], al1[0], ar1[0], b1[0]),
                  (W1[1], al1[1], ar1[1], b1[1])]
        for (W, al, ar, b) in layers:
            agg = _run_layer(np.ascontiguousarray(h), W, al, ar,
                             CH, srcidx, dstl)
            bsum = np.zeros((3, HID), np.float32)
            for e in range(9):
                bsum[DST_T[e]] += b[e]
            agg += bsum[:, None, :]
            h = _ln_relu(agg, gamma, beta)
        return np.einsum('tnh,tho->tno', h, Wc) + bc[:, None, :]
    except Exception:
        import traceback
        traceback.print_exc()
        return _kernel_np(x, edges, Wemb, bemb, W0, al0, ar0, b0, W1, al1,
                          ar1, b1, gamma, beta, Wc, bc)

